# revision 1
# baseline (speedup 1.0000x reference)
"""Trainium2 Bass kernel for nn_Ensembler (nms_detection).

Contract: kernel(**inputs) takes the FULL unsharded inputs
(voxel_logits [3,64,128,128,32] f32, query_logits [3,1,64,21] f32,
sem_prob_dense [21,128,128,32] f32) and returns the FULL output
[64,128,128,32] f32.

Strategy: shard the voxel grids over the flattened voxel dimension
N = X*Y*Z across 8 NeuronCores (each core owns a contiguous slice of
N).  The QxQ IoU statistics are computed as per-shard 0/1-mask GEMMs
(fp8 DoubleRow on the tensor engine) reduced with a tiny AllReduce;
the argmax / matching / merge / keep steps are then replicated on
every core, and the merge + keep + occupancy masking are
embarrassingly parallel over the local N slice.  The data-dependent
row gather aux_v[aux_idx] is realized as indirect DMAs that read the
aux logits from DRAM with device-computed row indices.

Numerical notes:
 - all mask decisions are computed from logit signs (exact): the
   iteration-2 anchor mask uses (sig(x0)+sig(x1))/2 > 0.5 <=>
   x0 + x1 > 0, avoiding sigmoid-LUT error in the decision path.
 - sigmoid LUT (ScalarE) max abs err ~3.6e-6 affects output values
   only.

Layouts per core (NS = 65536 voxels):
 - "n-layout": [128 part, ...] with n = p*512 + j (partition-major).
 - "q-layout": [128 part = (qb, q), T cols]: chunk ci covers
   n in [ci*2T, ci*2T+2T); rows 0:64 hold q for the first T, rows
   64:128 the second T.
 - L0 is read ONCE into a persistent q-layout SBUF tile that is
   overwritten in place by the merged anchor (pass B) and consumed by
   pass C.  Masks travel through DRAM as fp8 to switch layouts.
"""

import numpy as np

S = 3
Q = 64
X, Y, Z = 128, 128, 32
N = X * Y * Z           # 524288
C_SEM = 21
NCORES = 8
NS = N // NCORES        # 65536 voxels per core
JP = NS // 128          # 512 contiguous voxels per partition (n-layout)
T = 1024                # q-layout chunk free size
NCH = NS // (2 * T)     # 32 q-layout chunks
QC = 4                  # q rows per n-layout read chunk

_compiled = None


def _register_custom_dve_ops():
    """Register two fused DVE ops at runtime (halves the DVE op count on
    the blend/mask hot paths).  Purely additive registration in the
    concourse dve_ops tables; rows stay within the 5-bit byte-36 field."""
    import concourse.dve_ops as dve_ops
    from concourse.dve_ops import DveOp
    from concourse.dve_spec import (Spec, Src0, Src1, C0, C1, Zero, lower,
                                    _has_src1)
    from concourse.dve_uop import DveOpSpec

    if "ANT_BLEND2_K" in dve_ops._SUB_OPCODE_FOR_NAME:
        by = {op.name: op for op in dve_ops.OPS}
        return by["ANT_BLEND2_K"], by["ANT_MASKGT_K"]

    def make(name, spec):
        row = dve_ops._CUSTOM_DVE_ROW_BASE + len(dve_ops.OPS)
        assert row < 0x20
        dve_ops._SUB_OPCODE_FOR_NAME[name] = row
        shas = {}
        for ver in ("v3", "v4"):
            try:
                uops = lower(spec, ver=ver)
                shas[ver] = DveOpSpec(name=name, opcode=row, uops=uops,
                                      rd1_en=_has_src1(spec)).sha(ver)
            except Exception:
                pass
        op = DveOp(name, spec, subdim=False, uops_sha=shas)
        dve_ops.OPS.append(op)
        dve_ops.CUSTOM_DVE_SPECS[name] = spec
        return op

    blend2 = make("ANT_BLEND2_K", Spec(
        body=Src0 * C0 + Src1 * C1,
        reference=lambda in0, in1, s0, s1, imm2: (
            in0.astype(np.float32) * s0 + in1 * s1).astype(np.float32),
    ))
    maskgt = make("ANT_MASKGT_K", Spec(
        body=Zero < (Src0 + Src1 * C0),
        reference=lambda in0, in1, s0, s1, imm2: (
            (in0.astype(np.float32) + in1 * s0) > 0).astype(np.float32),
    ))
    return blend2, maskgt


def _build_program(phases=("A", "AR1", "B", "G2", "AR2", "C"), real_cc=True,
                   loop_k=None):
    import dataclasses
    import concourse.bass as bass
    import concourse.bacc as bacc
    import concourse.mybir as mybir
    import concourse.tile as tile

    phases = set(phases)
    dt = mybir.dt
    Alu = mybir.AluOpType
    Act = mybir.ActivationFunctionType
    DR = mybir.MatmulPerfMode.DoubleRow

    BLEND2, MASKGT = _register_custom_dve_ops()

    def dram_view(ap, pattern, offset_elems):
        """Raw [step,count] (element units) view of a DRAM tensor AP."""
        return dataclasses.replace(ap, ap=[list(p) for p in pattern],
                                   offset=offset_elems)

    nc = bacc.Bacc("TRN2", target_bir_lowering=False, debug=False,
                   num_devices=NCORES)

    l0 = nc.dram_tensor("l0", [Q, NS], dt.float32, kind="ExternalInput").ap()
    l1 = nc.dram_tensor("l1", [Q, NS], dt.float32, kind="ExternalInput").ap()
    l2 = nc.dram_tensor("l2", [Q, NS], dt.float32, kind="ExternalInput").ap()
    sem = nc.dram_tensor("sem", [C_SEM, NS], dt.float32,
                         kind="ExternalInput").ap()
    revcnt = nc.dram_tensor("revcnt", [Q, Q], dt.float32,
                            kind="ExternalInput").ap()
    iotap = nc.dram_tensor("iotap", [128, 1], dt.float32,
                           kind="ExternalInput").ap()
    out = nc.dram_tensor("out", [Q, NS], dt.float32,
                         kind="ExternalOutput").ap()

    import contextlib

    with tile.TileContext(nc) as tc:
        with (tc.For_i(0, loop_k, 1) if loop_k else
              contextlib.nullcontext()):
            _body(nc, tc, phases, real_cc, dram_view,
                  (l0, l1, l2, sem, revcnt, iotap, out), (BLEND2, MASKGT),
                  mybir)
    nc.compile()
    return nc


def _body(nc, tc, phases, real_cc, dram_view, tensors, custom_ops, mybir):
    import dataclasses
    import concourse.bass as bass

    dt = mybir.dt
    Alu = mybir.AluOpType
    Act = mybir.ActivationFunctionType
    DR = mybir.MatmulPerfMode.DoubleRow
    l0, l1, l2, sem, revcnt, iotap, out = tensors
    BLEND2, MASKGT = custom_ops

    if True:
        with tc.tile_pool(name="dram", bufs=1, space="DRAM") as dramp, \
             tc.tile_pool(name="psum", bufs=1, space="PSUM") as psump, \
             tc.tile_pool(name="stats", bufs=1) as stp:

            # ---- DRAM scratch ----------------------------------------
            m0_dram = dramp.tile([Q + 1, NS], dt.float8e4)
            ma2_dram = dramp.tile([Q + 1, NS], dt.float8e4)
            occ_dram = dramp.tile([1, NS], dt.float8e4)
            cc_in1 = dramp.tile([Q + 1, Q + 1], dt.float32)
            cc_out1 = dramp.tile([Q + 1, Q + 1], dt.float32)
            cc_in2 = dramp.tile([Q + 1, Q + 1], dt.float32)
            cc_out2 = dramp.tile([Q + 1, Q + 1], dt.float32)
            pack1_dram = dramp.tile([Q, 3], dt.float32)
            pack2_dram = dramp.tile([Q, 3], dt.float32)

            # ---- small persistent stat tiles -------------------------
            revc = stp.tile([Q, Q], dt.float32)
            nc.sync.dma_start(revc[:], revcnt[:])
            iou_a1 = stp.tile([Q, 1], dt.float32)
            iou_a2 = stp.tile([Q, 1], dt.float32)
            iotp = stp.tile([128, 1], dt.float32)
            nc.sync.dma_start(iotp[:], iotap[:])
            bd1 = stp.tile([128, 128], dt.float32)
            bd2 = stp.tile([128, 128], dt.float32)
            idxb_dram = dramp.tile([1, 2 * Q], dt.float32)
            idxb_dram2 = dramp.tile([1, 2 * Q], dt.float32)
            cb_pp = stp.tile([128, 3], dt.float32)   # [cb, matched1, 1-cb]
            c3k_pp = stp.tile([128, 3], dt.float32)  # [c3, keep, 1-c3]

            g1_ps = psump.tile([Q + 1, Q + 1], dt.float32)
            g2_ps = psump.tile([Q + 1, Q + 1], dt.float32)

            # big persistent region: holds L0 logits, then anchor2 in
            # place.  Split into 8 tiles so unit-level deps stay fine-
            # grained (one tile = 8 blend units of 512 cols).
            with tc.tile_pool(name="bigp", bufs=1) as bigp:
                l0q_tiles = []
                for b in range(8):
                    lt = bigp.tile([128, NS // 16], dt.float32,
                                   name=f"l0q_{b}")
                    l0q_tiles.append(lt)
                    for qb in range(2):
                        eng = nc.sync if (b + qb) % 2 == 0 else nc.scalar
                        eng.dma_start(
                            lt[qb * Q:(qb + 1) * Q, :],
                            dram_view(l0,
                                      [[NS, Q], [2 * T, 4], [1, T]],
                                      b * 4 * 2 * T + qb * T))

                def l0q_slice(u):
                    # unit u covers global cols [u*512, (u+1)*512)
                    ti, off = divmod(u * 512, NS // 16)
                    return l0q_tiles[ti][:, off:off + 512]

                # =====================================================
                # PASS A: m0 masks -> DRAM roundtrip; m1 (SBUF) -> G1;
                #         m2 masks kept in SBUF for G2
                # =====================================================
                with tc.tile_pool(name="m0p", bufs=1) as pa:
                    ones_c = pa.tile([128, JP], dt.float8e4)
                    nc.vector.memset(ones_c[:], 1.0)
                    nc.scalar.dma_start(
                        dram_view(m0_dram, [[JP, 128], [1, JP]], Q * NS),
                        ones_c[:])
                    # m0 masks from the q-layout L0 tiles -> m0_dram
                    for grp in range(8):
                        m0c = pa.tile([128, 4 * T], dt.float8e4, tag="m0c",
                                      bufs=2)
                        nc.vector.tensor_scalar(
                            m0c[:], l0q_tiles[grp][:], 0.0, None,
                            op0=Alu.is_gt)
                        for qb in range(2):
                            weng = nc.scalar if (grp + qb) % 2 == 0 else nc.sync
                            weng.dma_start(
                                dram_view(m0_dram,
                                          [[NS, Q], [2 * T, 4], [1, T]],
                                          grp * 8 * T + qb * T),
                                m0c[qb * Q:(qb + 1) * Q, :])
                    # m1 masks: n-layout direct to SBUF (j-major + ones col)
                    with tc.tile_pool(name="m1p", bufs=1) as pm1:
                        m1_sb = pm1.tile([128, JP, Q + 1], dt.float8e4)
                        nc.vector.memset(m1_sb[:, :, Q], 1.0)
                        for qc in range(Q // QC):
                            lc = pm1.tile([128, QC, JP], dt.float32,
                                          tag="ldchunk", bufs=2)
                            src = dram_view(l1,
                                            [[JP, 128], [NS, QC], [1, JP]],
                                            qc * QC * NS)
                            ldeng = nc.sync if qc % 2 == 0 else nc.scalar
                            ldeng.dma_start(lc[:], src)
                            nc.vector.tensor_scalar(
                                m1_sb[:, :, qc * QC:(qc + 1) * QC],
                                lc[:].rearrange("p q j -> p j q"), 0.0,
                                None, op0=Alu.is_gt)
                        # G1 GEMM: m0 readback (j-halves) x m1_sb, DoubleRow
                        for h in range(2):
                            m0t = pm1.tile([128, Q + 1, JP // 2],
                                           dt.float8e4, tag="m0t", bufs=1)
                            nc.sync.dma_start(
                                m0t[:],
                                dram_view(
                                    m0_dram,
                                    [[JP, 128], [NS, Q + 1], [1, JP // 2]],
                                    h * (JP // 2)))
                            for j in range(JP // 2):
                                gj = h * (JP // 2) + j
                                nc.tensor.matmul(
                                    g1_ps[:], lhsT=m0t[:, :, j],
                                    rhs=m1_sb[:, gj, :],
                                    start=(gj == 0), stop=(gj == JP - 1))

                # m2 masks: n-layout direct to SBUF, persists through G2
                pm2 = tc.alloc_tile_pool(name="m2p", bufs=1)
                m2_sb = pm2.tile([128, JP, Q + 1], dt.float8e4)
                nc.vector.memset(m2_sb[:, :, Q], 1.0)
                with tc.tile_pool(name="m2fill", bufs=1) as pmf:
                    for qc in range(Q // QC):
                        lc2 = pmf.tile([128, QC, JP], dt.float32,
                                       tag="ld2chunk", bufs=2)
                        src = dram_view(l2, [[JP, 128], [NS, QC], [1, JP]],
                                        qc * QC * NS)
                        ldeng = nc.scalar if qc % 2 == 0 else nc.sync
                        ldeng.dma_start(lc2[:], src)
                        nc.vector.tensor_scalar(
                            m2_sb[:, :, qc * QC:(qc + 1) * QC],
                            lc2[:].rearrange("p q j -> p j q"), 0.0,
                            None, op0=Alu.is_gt)

                # ---- shared stats machinery --------------------------
                def stats_round(g_ps, cc_in, cc_out, iou_a, bd, idx_dram):
                    sfx = cc_in.name
                    gs = stp.tile([Q + 1, Q + 1], dt.float32,
                                  name=f"gs_{sfx}")
                    nc.vector.tensor_copy(gs[:], g_ps[:])
                    nc.sync.dma_start(cc_in[:], gs[:])
                    if real_cc:
                        nc.gpsimd.collective_compute(
                            "AllReduce", Alu.add,
                            replica_groups=[list(range(NCORES))],
                            ins=[cc_in.opt()], outs=[cc_out.opt()])
                    else:
                        nc.sync.dma_start(cc_out[:], cc_in[:])
                    gr = stp.tile([Q + 1, Q + 1], dt.float32,
                                  name=f"gr_{sfx}")
                    nc.sync.dma_start(gr[:], cc_out[:])
                    sbb = stp.tile([Q, Q], dt.float32, name=f"sbb_{sfx}")
                    row = cc_out[Q:Q + 1, 0:Q]
                    nc.sync.dma_start(
                        sbb[:], dataclasses.replace(
                            row, ap=[[0, Q]] + [list(p) for p in row.ap[1:]]))
                    inter = gr[0:Q, 0:Q]
                    sa = gr[0:Q, Q:Q + 1]
                    u = stp.tile([Q, Q], dt.float32, name=f"u_{sfx}")
                    nc.vector.tensor_scalar(u[:], inter, sa, None,
                                            op0=Alu.subtract)
                    nc.vector.tensor_tensor(u[:], sbb[:], u[:],
                                            op=Alu.subtract)
                    nc.vector.tensor_scalar(u[:], u[:], 1.0, None,
                                            op0=Alu.max)
                    nc.vector.reciprocal(u[:], u[:])
                    iou = stp.tile([Q, Q], dt.float32, name=f"iou_{sfx}")
                    nc.vector.tensor_tensor(iou[:], inter, u[:], op=Alu.mult)
                    nc.vector.tensor_reduce(iou_a[:], iou[:],
                                            axis=mybir.AxisListType.X,
                                            op=Alu.max)
                    matched = stp.tile([Q, 1], dt.float32, name=f"mt_{sfx}")
                    nc.vector.tensor_scalar(matched[:], iou_a[:], 0.2, None,
                                            op0=Alu.is_gt)
                    eq = stp.tile([Q, Q], dt.float32, name=f"eq_{sfx}")
                    nc.vector.tensor_scalar(eq[:], iou[:], iou_a[:, 0:1],
                                            None, op0=Alu.is_equal)
                    nc.vector.tensor_tensor(eq[:], eq[:], revc[:],
                                            op=Alu.mult)
                    sm = stp.tile([Q, 1], dt.float32, name=f"sm_{sfx}")
                    nc.vector.tensor_reduce(sm[:], eq[:],
                                            axis=mybir.AxisListType.X,
                                            op=Alu.max)
                    nc.vector.tensor_scalar(sm[:], sm[:], -1.0, float(Q),
                                            op0=Alu.mult, op1=Alu.add)
                    # block-diagonal one-hot gather matrix:
                    # bd[p, m] = (idx[m % 64] + 64*(m//64) == p)
                    pkx = stp.tile([Q, 2], dt.float32, name=f"pkx_{sfx}")
                    nc.vector.tensor_copy(pkx[:, 0:1], sm[:])
                    nc.vector.tensor_scalar(pkx[:, 1:2], sm[:], 64.0, None,
                                            op0=Alu.add)
                    nc.sync.dma_start(
                        dram_view(idx_dram, [[1, Q], [Q, 2]], 0), pkx[:])
                    idxrow = stp.tile([128, 128], dt.float32,
                                      name=f"idxrow_{sfx}")
                    nc.sync.dma_start(
                        idxrow[:],
                        dram_view(idx_dram, [[0, 128], [1, 128]], 0))
                    nc.vector.tensor_scalar(bd[:], idxrow[:], iotp[:, 0:1],
                                            None, op0=Alu.is_equal)
                    return matched

                if "AR1" in phases:
                    matched1 = stats_round(g1_ps, cc_in1, cc_out1, iou_a1,
                                           bd1, idxb_dram)
                    cb64 = stp.tile([Q, 3], dt.float32)
                    nc.vector.tensor_scalar(cb64[:, 0:1], matched1[:], 0.5,
                                            None, op0=Alu.mult)
                    nc.vector.tensor_copy(cb64[:, 1:2], matched1[:])
                    nc.vector.tensor_scalar(cb64[:, 2:3], matched1[:], -0.5,
                                            1.0, op0=Alu.mult, op1=Alu.add)
                    nc.sync.dma_start(pack1_dram[:], cb64[:])
                    nc.sync.dma_start(
                        cb_pp[:],
                        dram_view(pack1_dram, [[0, 2], [3, Q], [1, 3]], 0))

                # =====================================================
                # PASS B: anchor2 blend in place + ma2 mask; G2 GEMM
                # =====================================================
                if "B" in phases:
                    with tc.tile_pool(name="blend", bufs=1) as pb:
                        ones_r = pb.tile([128, JP], dt.float8e4)
                        nc.vector.memset(ones_r[:], 1.0)
                        nc.scalar.dma_start(
                            dram_view(ma2_dram, [[JP, 128], [1, JP]],
                                      Q * NS),
                            ones_r[:])
                        for u in range(NS // 1024):   # 512-wide units
                            ci, hh = u // 2, u % 2
                            sl = l0q_slice(u)
                            l1c = pb.tile([128, 512], dt.float32,
                                          tag="l1c", bufs=4)
                            ldeng = nc.sync if u % 2 == 0 else nc.scalar
                            ldeng.dma_start(
                                l1c[:],
                                dram_view(l1,
                                          [[T, 2], [NS, Q], [1, 512]],
                                          ci * 2 * T + hh * 512))
                            # gather logits on PE: lg = blockdiag(sel1) @ l1c
                            lg = psump.tile([128, 512], dt.float32,
                                            tag="gps", bufs=2,
                                            name=f"lg_{u}")
                            nc.tensor.matmul(lg[:], lhsT=bd1[:], rhs=l1c[:],
                                             start=True, stop=True)
                            # exact mask (l0 + matched1*l1g) > 0 (logits!)
                            if u % 16 == 0:
                                ma2st = pb.tile([128, 8 * T], dt.float8e4,
                                                tag="ma2st", bufs=2)
                            nc.vector._custom_dve(
                                MASKGT,
                                out=ma2st[:, (u % 16) * 512:
                                          (u % 16 + 1) * 512],
                                in0=sl, in1=lg[:], s0=cb_pp[:, 1:2])
                            if u % 16 == 15:
                                grp = u // 16
                                for qb in range(2):
                                    weng = (nc.scalar if (grp + qb) % 2 == 0
                                            else nc.sync)
                                    weng.dma_start(
                                        dram_view(
                                            ma2_dram,
                                            [[NS, Q], [2 * T, 8], [1, T]],
                                            grp * 16 * T + qb * T),
                                        ma2st[qb * Q:(qb + 1) * Q, :])
                            p0c = pb.tile([128, 512], dt.float32, tag="p0c",
                                          bufs=2)
                            nc.scalar.activation(p0c[:], sl, Act.Sigmoid)
                            p1g = pb.tile([128, 512], dt.float32, tag="p1g",
                                          bufs=2)
                            nc.scalar.activation(p1g[:], lg[:], Act.Sigmoid)
                            # anchor2 = (1-cb)*p0 + cb*p1g, in place
                            nc.vector._custom_dve(
                                BLEND2, out=sl, in0=p0c[:], in1=p1g[:],
                                s0=cb_pp[:, 2:3], s1=cb_pp[:, 0:1])

                    if "G2" in phases:
                        with tc.tile_pool(name="g2", bufs=1) as pg:
                            ma2t = pg.tile([128, Q + 1, JP], dt.float8e4)
                            for g in range(8):
                                ps = slice(g * 16, (g + 1) * 16)
                                eng = nc.sync if g % 2 == 0 else nc.scalar
                                eng.dma_start(
                                    ma2t[ps, :, :],
                                    dram_view(
                                        ma2_dram,
                                        [[JP, 16], [NS, Q + 1], [1, JP]],
                                        g * 16 * JP))
                            for j in range(JP):
                                nc.tensor.matmul(
                                    g2_ps[:], lhsT=ma2t[:, :, j],
                                    rhs=m2_sb[:, j, :],
                                    start=(j == 0), stop=(j == JP - 1))
                    pm2.release()

                    if "AR2" in phases:
                        matched2 = stats_round(g2_ps, cc_in2, cc_out2,
                                               iou_a2, bd2, idxb_dram2)
                        pk = stp.tile([Q, 3], dt.float32)
                        nc.vector.tensor_scalar(pk[:, 0:1], matched2[:],
                                                1.0 / 3.0, None,
                                                op0=Alu.mult)
                        nc.vector.tensor_scalar(pk[:, 2:3], matched2[:],
                                                -1.0 / 3.0, 1.0,
                                                op0=Alu.mult, op1=Alu.add)
                        t64 = stp.tile([Q, 1], dt.float32)
                        nc.vector.tensor_tensor(t64[:], iou_a1[:],
                                                iou_a2[:], op=Alu.add)
                        nc.vector.tensor_scalar(pk[:, 1:2], t64[:], 0.5,
                                                0.2, op0=Alu.mult,
                                                op1=Alu.is_gt)
                        nc.sync.dma_start(pack2_dram[:], pk[:])
                        nc.sync.dma_start(
                            c3k_pp[:],
                            dram_view(pack2_dram, [[0, 2], [3, Q], [1, 3]],
                                      0))

                    # =================================================
                    # PASS C: final merge + keep + occupancy -> out
                    # =================================================
                    if "C" in phases:
                        with tc.tile_pool(name="passc", bufs=1) as pc:
                            # occupancy (overlaps the AR2 window):
                            # occ[n] = (max_{c>=1} sem[c,n] > sem[0,n])
                            sem0 = pc.tile([128, JP], dt.float32)
                            nc.sync.dma_start(
                                sem0[:],
                                dram_view(sem, [[JP, 128], [1, JP]], 0))
                            mx = pc.tile([128, JP], dt.float32)
                            nc.sync.dma_start(
                                mx[:],
                                dram_view(sem, [[JP, 128], [1, JP]], NS))
                            for g0 in range(2, C_SEM, 5):
                                rows = min(5, C_SEM - g0)
                                semc = pc.tile([128, 5, JP], dt.float32,
                                               tag="semc", bufs=1,
                                               name=f"semg{g0}")
                                nc.scalar.dma_start(
                                    semc[:, :rows, :],
                                    dram_view(sem,
                                              [[JP, 128], [NS, rows],
                                               [1, JP]],
                                              g0 * NS))
                                for k in range(rows):
                                    nc.vector.tensor_tensor(
                                        mx[:], mx[:], semc[:, k, :],
                                        op=Alu.max)
                            occ_n = pc.tile([128, JP], dt.float8e4)
                            nc.vector.tensor_tensor(occ_n[:], mx[:],
                                                    sem0[:], op=Alu.is_gt)
                            nc.sync.dma_start(
                                dram_view(occ_dram, [[JP, 128], [1, JP]],
                                          0),
                                occ_n[:])
                            occ_all = pc.tile([128, NS // 2], dt.float8e4)
                            for qb in range(2):
                                nc.scalar.dma_start(
                                    occ_all[qb * Q:(qb + 1) * Q, :],
                                    dram_view(
                                        occ_dram,
                                        [[0, Q], [2 * T, NCH], [1, T]],
                                        qb * T))
                            for u in range(NS // 1024):
                                ci, hh = u // 2, u % 2
                                a2s = l0q_slice(u)
                                l2c = pc.tile([128, 512], dt.float32,
                                              tag="l2c", bufs=4)
                                ldeng = nc.sync if u % 2 == 0 else nc.scalar
                                ldeng.dma_start(
                                    l2c[:],
                                    dram_view(l2,
                                              [[T, 2], [NS, Q], [1, 512]],
                                              ci * 2 * T + hh * 512))
                                lg2 = psump.tile([128, 512], dt.float32,
                                                 tag="gps", bufs=2,
                                                 name=f"lg2_{u}")
                                nc.tensor.matmul(lg2[:], lhsT=bd2[:],
                                                 rhs=l2c[:],
                                                 start=True, stop=True)
                                p2g = pc.tile([128, 512], dt.float32,
                                              tag="p2g", bufs=2)
                                nc.scalar.activation(p2g[:], lg2[:],
                                                     Act.Sigmoid)
                                sm2 = pc.tile([128, 512], dt.float32,
                                              tag="sm2", bufs=2)
                                nc.vector._custom_dve(
                                    BLEND2, out=sm2[:], in0=a2s,
                                    in1=p2g[:], s0=c3k_pp[:, 2:3],
                                    s1=c3k_pp[:, 0:1])
                                oc = pc.tile([128, 512], dt.float32,
                                             tag="oc", bufs=2)
                                nc.vector.scalar_tensor_tensor(
                                    oc[:], sm2[:], c3k_pp[:, 1:2],
                                    occ_all[:, u * 512:(u + 1) * 512],
                                    op0=Alu.mult, op1=Alu.mult)
                                weng = nc.sync if u % 2 == 0 else nc.scalar
                                weng.dma_start(
                                    dram_view(out,
                                              [[T, 2], [NS, Q], [1, 512]],
                                              ci * 2 * T + hh * 512),
                                    oc[:])

                if "B" not in phases:
                    pm2.release()
            if "C" not in phases:
                nc.sync.dma_start(
                    dram_view(out, [[NS, Q], [1, Q]], 0), revc[:])


def _get_program():
    global _compiled
    if _compiled is None:
        _compiled = _build_program()
    return _compiled


def _make_in_maps(voxel_logits, sem_prob_dense):
    vl = np.ascontiguousarray(
        np.asarray(voxel_logits, dtype=np.float32).reshape(S, Q, N))
    sp = np.ascontiguousarray(
        np.asarray(sem_prob_dense, dtype=np.float32).reshape(C_SEM, N))
    revcnt = np.tile((Q - np.arange(Q, dtype=np.float32))[None, :], (Q, 1))
    iotap = np.arange(128, dtype=np.float32)[:, None]
    in_maps = []
    for c in range(NCORES):
        sl = slice(c * NS, (c + 1) * NS)
        in_maps.append({
            "l0": np.ascontiguousarray(vl[0, :, sl]),
            "l1": np.ascontiguousarray(vl[1, :, sl]),
            "l2": np.ascontiguousarray(vl[2, :, sl]),
            "sem": np.ascontiguousarray(sp[:, sl]),
            "revcnt": revcnt,
            "iotap": iotap,
        })
    return in_maps


def profile_run(inputs):
    """Run once with NTFF tracing; returns exec_time_ns or None."""
    from concourse.bass_utils import run_bass_kernel_spmd

    nc = _get_program()
    in_maps = _make_in_maps(inputs["voxel_logits"], inputs["sem_prob_dense"])
    res = run_bass_kernel_spmd(nc, in_maps, list(range(NCORES)), trace=True)
    return res.exec_time_ns


def kernel(voxel_logits, query_logits, sem_prob_dense):
    from concourse.bass_utils import run_bass_kernel_spmd

    nc = _get_program()
    in_maps = _make_in_maps(voxel_logits, sem_prob_dense)
    res = run_bass_kernel_spmd(nc, in_maps, list(range(NCORES)))
    full = np.concatenate([res.results[c]["out"] for c in range(NCORES)],
                          axis=1)
    return full.reshape(Q, X, Y, Z).astype(np.float32)



# revision 9
# speedup vs baseline: 1.5780x; 1.5780x over previous
"""Trainium2 Bass kernel for nn_Ensembler (nms_detection).

Contract: kernel(**inputs) takes the FULL unsharded inputs
(voxel_logits [3,64,128,128,32] f32, query_logits [3,1,64,21] f32,
sem_prob_dense [21,128,128,32] f32) and returns the FULL output
[64,128,128,32] f32.

Strategy: shard over the flattened voxel dimension N = X*Y*Z across 8
NeuronCores.  Mask decisions for rounds use logit signs; round-2's
blended-anchor mask needs BIT-EXACT f32 sums (the round-2 IoU argmax
has a 5e-7 near-tie), so l0 is streamed in f32 and the matched aux
rows l1[idx1] are fetched bit-exact with indirect DMAs from a
chunk-major f32 copy.  Everything value-only (m0/m1/m2 masks, sigmoid
probs, blends, output) runs from bf16 staging, halving that traffic.
Masks travel q-layout -> DRAM fp8 -> n-layout for the QxQ IoU GEMMs
(fp8 on the PE); the [65,65] partials are AllReduced (2 rounds).
keep is folded into the final blend coefficients; occ multiplies the
blended output via a broadcast-streamed fp8 tile.  Output is written
bf16 and upcast on host.

Layouts per core (NS = 65536 voxels):
 - q-layout group tile g (8 groups): [128 part = (qb, q), 4096 cols],
   col = k*1024 + t  <->  n = (4g+k)*2048 + qb*1024 + t.
 - n-layout mask readback: [128 part = n>>9, 65 q, J j], n = p*512+j.
 - l1 chunk-major f32: [64, Q, 1024], chunk c = n>>10; the gather
   index per partition (qb,q) is qb*64 + idx1[q], coef 1024.
"""

import numpy as np

S = 3
Q = 64
X, Y, Z = 128, 128, 32
N = X * Y * Z           # 524288
C_SEM = 21
NCORES = 8
NS = N // NCORES        # 65536 voxels per core
JP = NS // 128          # 512 (n-layout cols per partition)
T = 1024                # q-layout half-chunk width
NCH = NS // (2 * T)     # 32 chunks
NG = 8                  # q-layout groups (4 chunks each)
GC = 4096               # cols per group tile

_compiled = None


def _register_custom_dve_ops():
    """BLEND2: out = in0*s0 + in1*s1.  MASKGT: out = (in0 + in1*s0) > 0."""
    import concourse.dve_ops as dve_ops
    from concourse.dve_ops import DveOp
    from concourse.dve_spec import (Spec, Src0, Src1, C0, C1, Zero, lower,
                                    _has_src1)
    from concourse.dve_uop import DveOpSpec

    if "ANT_BLEND2_K" in dve_ops._SUB_OPCODE_FOR_NAME:
        by = {op.name: op for op in dve_ops.OPS}
        return by["ANT_BLEND2_K"], by["ANT_MASKGT_K"]

    def make(name, spec):
        row = dve_ops._CUSTOM_DVE_ROW_BASE + len(dve_ops.OPS)
        assert row < 0x20
        dve_ops._SUB_OPCODE_FOR_NAME[name] = row
        shas = {}
        for ver in ("v3", "v4"):
            try:
                uops = lower(spec, ver=ver)
                shas[ver] = DveOpSpec(name=name, opcode=row, uops=uops,
                                      rd1_en=_has_src1(spec)).sha(ver)
            except Exception:
                pass
        op = DveOp(name, spec, subdim=False, uops_sha=shas)
        dve_ops.OPS.append(op)
        dve_ops.CUSTOM_DVE_SPECS[name] = spec
        return op

    blend2 = make("ANT_BLEND2_K", Spec(
        body=Src0 * C0 + Src1 * C1,
        reference=lambda in0, in1, s0, s1, imm2: (
            in0.astype(np.float32) * s0 + in1 * s1).astype(np.float32),
    ))
    maskgt = make("ANT_MASKGT_K", Spec(
        body=Zero < (Src0 + Src1 * C0),
        reference=lambda in0, in1, s0, s1, imm2: (
            (in0.astype(np.float32) + in1 * s0) > 0).astype(np.float32),
    ))
    return blend2, maskgt


def _build_program(real_cc=True):
    import dataclasses
    import concourse.bass as bass
    import concourse.bacc as bacc
    import concourse.mybir as mybir
    import concourse.tile as tile

    dt = mybir.dt
    BLEND2, MASKGT = _register_custom_dve_ops()

    def dram_view(ap, pattern, offset_elems):
        return dataclasses.replace(ap, ap=[list(p) for p in pattern],
                                   offset=offset_elems)

    nc = bacc.Bacc("TRN2", target_bir_lowering=False, debug=False,
                   num_devices=NCORES)

    l0f = nc.dram_tensor("l0f", [Q, NS], dt.float32,
                         kind="ExternalInput").ap()
    l0b = nc.dram_tensor("l0b", [Q, NS], dt.bfloat16,
                         kind="ExternalInput").ap()
    l1b = nc.dram_tensor("l1b", [Q, NS], dt.bfloat16,
                         kind="ExternalInput").ap()
    l1fc = nc.dram_tensor("l1fc", [NS // T, Q, T], dt.float32,
                          kind="ExternalInput").ap()
    l2b = nc.dram_tensor("l2b", [Q, NS], dt.bfloat16,
                         kind="ExternalInput").ap()
    sem = nc.dram_tensor("sem", [C_SEM, NS], dt.float32,
                         kind="ExternalInput").ap()
    revcnt = nc.dram_tensor("revcnt", [Q, Q], dt.float32,
                            kind="ExternalInput").ap()
    iotap = nc.dram_tensor("iotap", [128, 1], dt.float32,
                           kind="ExternalInput").ap()
    out = nc.dram_tensor("out", [Q, NS], dt.bfloat16,
                         kind="ExternalOutput").ap()

    with tile.TileContext(nc) as tc:
        _body(nc, tc, bass, real_cc, dram_view,
              (l0f, l0b, l1b, l1fc, l2b, sem, revcnt, iotap, out),
              (BLEND2, MASKGT), mybir)
    nc.compile()
    return nc


def _qpat(g, qb):
    """q-layout DMA pattern for group g half qb on a [Q, NS] DRAM tensor.
    Pairs with tile rows [qb*64:(qb+1)*64] of a [128, 4096] group tile."""
    return [[NS, Q], [2 * T, 4], [1, T]], g * 8 * T + qb * T


def _body(nc, tc, bass, real_cc, dram_view, tensors, custom_ops, mybir):
    import dataclasses

    dt = mybir.dt
    Alu = mybir.AluOpType
    Act = mybir.ActivationFunctionType
    l0f, l0b, l1b, l1fc, l2b, sem, revcnt, iotap, out = tensors
    BLEND2, MASKGT = custom_ops

    with tc.tile_pool(name="dram", bufs=1, space="DRAM") as dramp, \
         tc.tile_pool(name="psum", bufs=1, space="PSUM") as psump, \
         tc.tile_pool(name="stats", bufs=1) as stp:

        # ---- DRAM scratch ----------------------------------------
        m0d = dramp.tile([Q + 1, NS], dt.float8e4)
        m1d = dramp.tile([Q + 1, NS], dt.float8e4)
        m2d = dramp.tile([Q + 1, NS], dt.float8e4)
        ma2d = dramp.tile([Q + 1, NS], dt.float8e4)
        occ_dram = dramp.tile([1, NS], dt.float8e4)
        cc_in1 = dramp.tile([Q + 1, Q + 1], dt.float32)
        cc_out1 = dramp.tile([Q + 1, Q + 1], dt.float32)
        cc_in2 = dramp.tile([Q + 1, Q + 1], dt.float32)
        cc_out2 = dramp.tile([Q + 1, Q + 1], dt.float32)
        pack1_dram = dramp.tile([Q, 3], dt.float32)
        pack2_dram = dramp.tile([Q, 3], dt.float32)
        idx1_dram = dramp.tile([1, Q], dt.float32)
        idx2_dram = dramp.tile([1, 2 * Q], dt.float32)

        # ---- small persistent stat tiles -------------------------
        revc = stp.tile([Q, Q], dt.float32)
        nc.sync.dma_start(revc[:], revcnt[:])
        iou_a1 = stp.tile([Q, 1], dt.float32)
        iou_a2 = stp.tile([Q, 1], dt.float32)
        iotp = stp.tile([128, 1], dt.float32)
        nc.sync.dma_start(iotp[:], iotap[:])
        bd2 = stp.tile([128, 128], dt.bfloat16)
        cb_pp = stp.tile([128, 3], dt.float32)   # [cb, matched1, 1-cb]
        c3k_pp = stp.tile([128, 3], dt.float32)  # [k*c3, keep, k*(1-c3)]
        idx_i32 = stp.tile([128, 1], dt.int32)

        g1_ps = psump.tile([Q + 1, Q + 1], dt.float32)
        g2_ps = psump.tile([Q + 1, Q + 1], dt.float32)

        # ones rows (q = Q) of all four mask tensors, written once
        with tc.tile_pool(name="ones", bufs=1) as po:
            onesc = po.tile([128, JP], dt.float8e4)
            nc.vector.memset(onesc[:], 1.0)
            for md in (m0d, m1d, m2d, ma2d):
                nc.scalar.dma_start(
                    dram_view(md, [[JP, 128], [1, JP]], Q * NS), onesc[:])

        # anchor2 (bf16) persistent pool -- bottom of the big stack
        pAnch = tc.alloc_tile_pool(name="anch", bufs=1)
        anch = [pAnch.tile([128, GC], dt.bfloat16, name=f"an{g}")
                for g in range(NG)]

        # ===== Phase A: masks from bf16 reads, G1 pipelined =======
        # l0/l1 group loads interleave so both masks for the first half
        # of n are in DRAM at 50%; G1 streams n-window halves
        # (n = h*32768 + p*256 + j) and starts on half 0 early.
        with tc.tile_pool(name="mstA", bufs=1) as pma, \
             tc.tile_pool(name="g1m", bufs=1) as pg1:
            def g1_half(h):
                m0t = pg1.tile([128, Q + 1, 256], dt.float8e4, tag="m0t",
                               bufs=2)
                m1t = pg1.tile([128, Q + 1, 256], dt.float8e4, tag="m1t",
                               bufs=2)
                nc.sync.dma_start(
                    m0t[:],
                    dram_view(m0d, [[256, 128], [NS, Q + 1], [1, 256]],
                              h * 32768))
                nc.scalar.dma_start(
                    m1t[:],
                    dram_view(m1d, [[256, 128], [NS, Q + 1], [1, 256]],
                              h * 32768))
                for j in range(256):
                    gj = h * 256 + j
                    nc.tensor.matmul(g1_ps[:], lhsT=m0t[:, :, j],
                                     rhs=m1t[:, :, j],
                                     start=(gj == 0), stop=(gj == JP - 1))

            for g in range(NG):
                for src, md, tg in ((l0b, m0d, "0"), (l1b, m1d, "1")):
                    lt = pma.tile([128, GC], dt.bfloat16, tag="ld" + tg,
                                  bufs=3)
                    for qb in range(2):
                        pat, off = _qpat(g, qb)
                        nc.sync.dma_start(lt[qb * Q:(qb + 1) * Q, :],
                                          dram_view(src, pat, off))
                    mk = pma.tile([128, GC], dt.float8e4, tag="mk" + tg,
                                  bufs=2)
                    nc.vector.tensor_scalar(mk[:], lt[:], 0.0, None,
                                            op0=Alu.is_gt)
                    for qb in range(2):
                        pat, off = _qpat(g, qb)
                        nc.scalar.dma_start(dram_view(md, pat, off),
                                            mk[qb * Q:(qb + 1) * Q, :])
                if g == 3:
                    g1_half(0)
            g1_half(1)

        # ===== AR1 ================================================
        gs1 = stp.tile([Q + 1, Q + 1], dt.float32, name="gs1")
        nc.vector.tensor_copy(gs1[:], g1_ps[:])
        nc.sync.dma_start(cc_in1[:], gs1[:])
        if real_cc:
            nc.gpsimd.collective_compute(
                "AllReduce", Alu.add,
                replica_groups=[list(range(NCORES))],
                ins=[cc_in1.opt()], outs=[cc_out1.opt()])
        else:
            nc.sync.dma_start(cc_out1[:], cc_in1[:])

        # ---- AR1 overlap: occupancy bit per voxel ----------------
        with tc.tile_pool(name="occc", bufs=1) as pc:
            sem0 = pc.tile([128, JP], dt.float32)
            nc.scalar.dma_start(sem0[:],
                                dram_view(sem, [[JP, 128], [1, JP]], 0))
            mx = pc.tile([128, JP], dt.float32)
            nc.scalar.dma_start(mx[:],
                                dram_view(sem, [[JP, 128], [1, JP]], NS))
            for g0 in range(2, C_SEM, 5):
                rows = min(5, C_SEM - g0)
                semc = pc.tile([128, 5, JP], dt.float32, tag="semc", bufs=2,
                               name=f"semg{g0}")
                nc.scalar.dma_start(
                    semc[:, :rows, :],
                    dram_view(sem, [[JP, 128], [NS, rows], [1, JP]],
                              g0 * NS))
                for k in range(rows):
                    nc.vector.tensor_tensor(mx[:], mx[:], semc[:, k, :],
                                            op=Alu.max)
            occ_n = pc.tile([128, JP], dt.float8e4)
            nc.vector.tensor_tensor(occ_n[:], mx[:], sem0[:], op=Alu.is_gt)
            nc.scalar.dma_start(
                dram_view(occ_dram, [[JP, 128], [1, JP]], 0), occ_n[:])

        # ---- AR1 stats tail --------------------------------------
        gr1 = stp.tile([Q + 1, Q + 1], dt.float32, name="gr1")
        nc.sync.dma_start(gr1[:], cc_out1[:])
        sbb1 = stp.tile([Q, Q], dt.float32, name="sbb1")
        row = cc_out1[Q:Q + 1, 0:Q]
        nc.sync.dma_start(
            sbb1[:], dataclasses.replace(
                row, ap=[[0, Q]] + [list(p) for p in row.ap[1:]]))
        matched1, sm1 = _stats_tail(nc, stp, dram_view, mybir, gr1, sbb1,
                                    revc, iou_a1, "r1")
        # indirect-gather index: qb*64 + idx1[q] per partition
        nc.sync.dma_start(dram_view(idx1_dram, [[1, Q], [1, 1]], 0), sm1[:])
        idxf = stp.tile([128, 1], dt.float32, name="idxf")
        nc.sync.dma_start(idxf[:],
                          dram_view(idx1_dram, [[0, 2], [1, Q], [1, 1]], 0))
        qbv = stp.tile([128, 1], dt.float32, name="qbv")
        nc.vector.tensor_scalar(qbv[:], iotp[:], 63.5, None, op0=Alu.is_gt)
        idxi_f = stp.tile([128, 1], dt.float32, name="idxi_f")
        nc.vector.scalar_tensor_tensor(idxi_f[:], qbv[:], 64.0, idxf[:],
                                       op0=Alu.mult, op1=Alu.add)
        nc.vector.tensor_copy(idx_i32[:], idxi_f[:])
        # blend coefficients: cb = matched1/2
        cb64 = stp.tile([Q, 3], dt.float32)
        nc.vector.tensor_scalar(cb64[:, 0:1], matched1[:], 0.5, None,
                                op0=Alu.mult)
        nc.vector.tensor_copy(cb64[:, 1:2], matched1[:])
        nc.vector.tensor_scalar(cb64[:, 2:3], matched1[:], -0.5, 1.0,
                                op0=Alu.mult, op1=Alu.add)
        nc.sync.dma_start(pack1_dram[:], cb64[:])
        nc.sync.dma_start(
            cb_pp[:], dram_view(pack1_dram, [[0, 2], [3, Q], [1, 3]], 0))

        # ===== Pass B: f32 stream + indirect gather + blend =======
        with tc.tile_pool(name="passb", bufs=1) as pb:
            for g in range(NG):
                ma2st = pb.tile([128, GC], dt.float8e4, tag="ma2st", bufs=2)
                for k in range(4):
                    u = 4 * g + k
                    l0u = pb.tile([128, T], dt.float32, tag="l0u", bufs=8)
                    nc.sync.dma_start(
                        l0u[:],
                        dram_view(l0f, [[T, 2], [NS, Q], [1, T]], u * 2 * T))
                    gat = pb.tile([128, T], dt.float32, tag="gat", bufs=4)
                    nc.gpsimd.indirect_dma_start(
                        out=gat[:], out_offset=None,
                        in_=dram_view(l1fc, [[T, 2 * Q], [1, T]], 0),
                        in_offset=bass.IndirectOffsetOnAxis(
                            ap=idx_i32[:, 0:1], axis=0),
                        element_offset=u * 2 * Q * T)
                    # exact round-2 anchor mask: (l0 + matched1*l1g) > 0
                    nc.vector._custom_dve(
                        MASKGT, out=ma2st[:, k * T:(k + 1) * T],
                        in0=l0u[:], in1=gat[:], s0=cb_pp[:, 1:2])
                    p0u = pb.tile([128, T], dt.bfloat16, tag="p0u", bufs=4)
                    nc.scalar.activation(p0u[:], l0u[:], Act.Sigmoid)
                    p1g = pb.tile([128, T], dt.bfloat16, tag="p1g", bufs=4)
                    nc.scalar.activation(p1g[:], gat[:], Act.Sigmoid)
                    nc.vector._custom_dve(
                        BLEND2, out=anch[g][:, k * T:(k + 1) * T],
                        in0=p0u[:], in1=p1g[:],
                        s0=cb_pp[:, 2:3], s1=cb_pp[:, 0:1])
                for qb in range(2):
                    pat, off = _qpat(g, qb)
                    nc.scalar.dma_start(dram_view(ma2d, pat, off),
                                        ma2st[qb * Q:(qb + 1) * Q, :])

        # ===== l2 loads + m2 masks + sigmoid ======================
        pC = tc.alloc_tile_pool(name="bigC", bufs=1)
        c_tiles = [pC.tile([128, GC], dt.bfloat16, name=f"c{g}")
                   for g in range(NG)]
        with tc.tile_pool(name="mst2", bufs=1) as pm2:
            for g in range(NG):
                for qb in range(2):
                    pat, off = _qpat(g, qb)
                    nc.sync.dma_start(c_tiles[g][qb * Q:(qb + 1) * Q, :],
                                      dram_view(l2b, pat, off))
                mk = pm2.tile([128, GC], dt.float8e4, tag="mk2", bufs=2)
                nc.vector.tensor_scalar(mk[:], c_tiles[g][:], 0.0, None,
                                        op0=Alu.is_gt)
                for qb in range(2):
                    pat, off = _qpat(g, qb)
                    nc.scalar.dma_start(dram_view(m2d, pat, off),
                                        mk[qb * Q:(qb + 1) * Q, :])
                nc.scalar.activation(c_tiles[g][:], c_tiles[g][:],
                                     Act.Sigmoid)

        # ===== G2 GEMM (quarter chunks: tighter SBUF here) ========
        with tc.tile_pool(name="g2m", bufs=1) as pg2:
            for h in range(4):
                ma2t = pg2.tile([128, Q + 1, 128], dt.float8e4, tag="ma2t",
                                bufs=2)
                m2t = pg2.tile([128, Q + 1, 128], dt.float8e4, tag="m2t",
                               bufs=2)
                nc.sync.dma_start(
                    ma2t[:],
                    dram_view(ma2d, [[JP, 128], [NS, Q + 1], [1, 128]],
                              h * 128))
                nc.scalar.dma_start(
                    m2t[:], dram_view(m2d, [[JP, 128], [NS, Q + 1], [1, 128]],
                                      h * 128))
                for j in range(128):
                    gj = h * 128 + j
                    nc.tensor.matmul(g2_ps[:], lhsT=ma2t[:, :, j],
                                     rhs=m2t[:, :, j],
                                     start=(gj == 0), stop=(gj == JP - 1))

        # ===== AR2 ================================================
        gs2 = stp.tile([Q + 1, Q + 1], dt.float32, name="gs2")
        nc.vector.tensor_copy(gs2[:], g2_ps[:])
        nc.sync.dma_start(cc_in2[:], gs2[:])
        if real_cc:
            nc.gpsimd.collective_compute(
                "AllReduce", Alu.add,
                replica_groups=[list(range(NCORES))],
                ins=[cc_in2.opt()], outs=[cc_out2.opt()])
        else:
            nc.sync.dma_start(cc_out2[:], cc_in2[:])

        # ---- AR2 overlap: stream occ broadcast tiles -------------
        occp = tc.alloc_tile_pool(name="occp", bufs=1)
        occ_st = [occp.tile([128, GC], dt.float8e4, name=f"oc{g}")
                  for g in range(NG)]
        for g in range(NG):
            for qb in range(2):
                eng = nc.scalar if (g + qb) % 2 == 0 else nc.sync
                eng.dma_start(
                    occ_st[g][qb * Q:(qb + 1) * Q, :],
                    dram_view(occ_dram, [[0, Q], [2 * T, 4], [1, T]],
                              g * 8 * T + qb * T))

        # ---- AR2 stats tail --------------------------------------
        gr2 = stp.tile([Q + 1, Q + 1], dt.float32, name="gr2")
        nc.sync.dma_start(gr2[:], cc_out2[:])
        sbb2 = stp.tile([Q, Q], dt.float32, name="sbb2")
        row = cc_out2[Q:Q + 1, 0:Q]
        nc.sync.dma_start(
            sbb2[:], dataclasses.replace(
                row, ap=[[0, Q]] + [list(p) for p in row.ap[1:]]))
        matched2, sm2 = _stats_tail(nc, stp, dram_view, mybir, gr2, sbb2,
                                    revc, iou_a2, "r2")
        # bd2 one-hot gather matrix from idx2 (block-diagonal)
        pkx = stp.tile([Q, 2], dt.float32, name="pkx2")
        nc.vector.tensor_copy(pkx[:, 0:1], sm2[:])
        nc.vector.tensor_scalar(pkx[:, 1:2], sm2[:], 64.0, None, op0=Alu.add)
        nc.sync.dma_start(dram_view(idx2_dram, [[1, Q], [Q, 2]], 0), pkx[:])
        idxrow = stp.tile([128, 128], dt.float32, name="idxrow2")
        nc.sync.dma_start(idxrow[:],
                          dram_view(idx2_dram, [[0, 128], [1, 128]], 0))
        nc.vector.tensor_scalar(bd2[:], idxrow[:], iotp[:, 0:1], None,
                                op0=Alu.is_equal)
        # final coefficients: c3 = matched2/3, keep folded in
        pk = stp.tile([Q, 3], dt.float32)
        t64 = stp.tile([Q, 1], dt.float32)
        nc.vector.tensor_tensor(t64[:], iou_a1[:], iou_a2[:], op=Alu.add)
        keep = stp.tile([Q, 1], dt.float32)
        nc.vector.tensor_scalar(keep[:], t64[:], 0.5, 0.2, op0=Alu.mult,
                                op1=Alu.is_gt)
        c3 = stp.tile([Q, 1], dt.float32)
        nc.vector.tensor_scalar(c3[:], matched2[:], 1.0 / 3.0, None,
                                op0=Alu.mult)
        nc.vector.tensor_tensor(pk[:, 0:1], keep[:], c3[:], op=Alu.mult)
        nc.vector.tensor_copy(pk[:, 1:2], keep[:])
        nc.vector.tensor_scalar(c3[:], c3[:], -1.0, 1.0, op0=Alu.mult,
                                op1=Alu.add)
        nc.vector.tensor_tensor(pk[:, 2:3], keep[:], c3[:], op=Alu.mult)
        nc.sync.dma_start(pack2_dram[:], pk[:])
        nc.sync.dma_start(
            c3k_pp[:], dram_view(pack2_dram, [[0, 2], [3, Q], [1, 3]], 0))

        # ===== Pass C: gather + final blend * occ -> out ==========
        H = T // 2
        with tc.tile_pool(name="passc", bufs=1) as pcc:
            for g in range(NG):
                for k in range(8):
                    sl = slice(k * H, (k + 1) * H)
                    lg2 = psump.tile([128, H], dt.float32, tag="gps", bufs=2,
                                     name=f"lg2_{g}_{k}")
                    nc.tensor.matmul(lg2[:], lhsT=bd2[:],
                                     rhs=c_tiles[g][:, sl],
                                     start=True, stop=True)
                    p2g = pcc.tile([128, H], dt.bfloat16, tag="p2g", bufs=4)
                    nc.scalar.activation(p2g[:], lg2[:], Act.Copy)
                    ob = pcc.tile([128, H], dt.bfloat16, tag="ob", bufs=4)
                    nc.vector._custom_dve(
                        BLEND2, out=ob[:], in0=anch[g][:, sl], in1=p2g[:],
                        s0=c3k_pp[:, 2:3], s1=c3k_pp[:, 0:1])
                    oc = pcc.tile([128, H], dt.bfloat16, tag="oc", bufs=4)
                    nc.vector.tensor_tensor(
                        oc[:], ob[:], occ_st[g][:, sl], op=Alu.mult)
                    u = 4 * g + k // 2
                    weng = nc.sync if k % 2 == 0 else nc.scalar
                    weng.dma_start(
                        dram_view(out, [[T, 2], [NS, Q], [1, H]],
                                  u * 2 * T + (k % 2) * H),
                        oc[:])
        occp.release()
        pC.release()
        pAnch.release()


def _stats_tail(nc, stp, dram_view, mybir, gr, sbb, revc, iou_a, sfx):
    """IoU stats from the AllReduced [Q+1,Q+1] grid.  Returns (matched,
    sm) where sm is the first-argmax aux index as f32 [Q,1]."""
    dt = mybir.dt
    Alu = mybir.AluOpType
    inter = gr[0:Q, 0:Q]
    sa = gr[0:Q, Q:Q + 1]
    u = stp.tile([Q, Q], dt.float32, name=f"u_{sfx}")
    nc.vector.tensor_scalar(u[:], inter, sa, None, op0=Alu.subtract)
    nc.vector.tensor_tensor(u[:], sbb[:], u[:], op=Alu.subtract)
    nc.vector.tensor_scalar(u[:], u[:], 1.0, None, op0=Alu.max)
    nc.vector.reciprocal(u[:], u[:])
    iou = stp.tile([Q, Q], dt.float32, name=f"iou_{sfx}")
    nc.vector.tensor_tensor(iou[:], inter, u[:], op=Alu.mult)
    nc.vector.tensor_reduce(iou_a[:], iou[:], axis=mybir.AxisListType.X,
                            op=Alu.max)
    matched = stp.tile([Q, 1], dt.float32, name=f"mt_{sfx}")
    nc.vector.tensor_scalar(matched[:], iou_a[:], 0.2, None, op0=Alu.is_gt)
    eq = stp.tile([Q, Q], dt.float32, name=f"eq_{sfx}")
    nc.vector.tensor_scalar(eq[:], iou[:], iou_a[:, 0:1], None,
                            op0=Alu.is_equal)
    nc.vector.tensor_tensor(eq[:], eq[:], revc[:], op=Alu.mult)
    sm = stp.tile([Q, 1], dt.float32, name=f"sm_{sfx}")
    nc.vector.tensor_reduce(sm[:], eq[:], axis=mybir.AxisListType.X,
                            op=Alu.max)
    nc.vector.tensor_scalar(sm[:], sm[:], -1.0, float(Q), op0=Alu.mult,
                            op1=Alu.add)
    return matched, sm


def _get_program():
    global _compiled
    if _compiled is None:
        _compiled = _build_program()
    return _compiled


def _make_in_maps(voxel_logits, sem_prob_dense):
    import ml_dtypes
    bf16 = ml_dtypes.bfloat16
    vl = np.asarray(voxel_logits, dtype=np.float32).reshape(S, Q, N)
    sp = np.ascontiguousarray(
        np.asarray(sem_prob_dense, dtype=np.float32).reshape(C_SEM, N))
    revcnt = np.tile((Q - np.arange(Q, dtype=np.float32))[None, :], (Q, 1))
    iotap = np.arange(128, dtype=np.float32)[:, None]
    in_maps = []
    for c in range(NCORES):
        sl = slice(c * NS, (c + 1) * NS)
        l1f = vl[1, :, sl]
        in_maps.append({
            "l0f": np.ascontiguousarray(vl[0, :, sl]),
            "l0b": np.ascontiguousarray(vl[0, :, sl].astype(bf16)),
            "l1b": np.ascontiguousarray(l1f.astype(bf16)),
            "l1fc": np.ascontiguousarray(
                l1f.reshape(Q, NS // T, T).transpose(1, 0, 2)),
            "l2b": np.ascontiguousarray(vl[2, :, sl].astype(bf16)),
            "sem": np.ascontiguousarray(sp[:, sl]),
            "revcnt": revcnt,
            "iotap": iotap,
        })
    return in_maps


def profile_run(inputs):
    """Run once with NTFF tracing; returns exec_time_ns or None."""
    from concourse.bass_utils import run_bass_kernel_spmd

    nc = _get_program()
    in_maps = _make_in_maps(inputs["voxel_logits"], inputs["sem_prob_dense"])
    res = run_bass_kernel_spmd(nc, in_maps, list(range(NCORES)), trace=True)
    return res.exec_time_ns


def kernel(voxel_logits, query_logits, sem_prob_dense):
    from concourse.bass_utils import run_bass_kernel_spmd

    nc = _get_program()
    in_maps = _make_in_maps(voxel_logits, sem_prob_dense)
    res = run_bass_kernel_spmd(nc, in_maps, list(range(NCORES)))
    full = np.concatenate([res.results[c]["out"] for c in range(NCORES)],
                          axis=1)
    return full.reshape(Q, X, Y, Z).astype(np.float32)
